# revision 57
# baseline (speedup 1.0000x reference)
"""Trainium2 Bass kernel for nn_AttentionTanh (B=8, S=2048, F=1024, U=256).

Data-parallel over batch: each of the 8 NeuronCores computes the full
attention for one batch example. No collectives.

Per-core dataflow (all matmuls via TensorE, out = lhsT.T @ rhs):
  xT   [F, S]  (host-swizzled bf16 input shard, F on partitions)
  QT   [u, s] = tanh(Wq.T @ x.T)  -> matmul(lhsT=Wq[f,u], rhs=xT[f,s])
  KT   [u, s] = tanh(Wk.T @ x.T)      QT/KT stored fp8e4 (scores run in
                                      fp8 DoubleRow; tanh bounds |q|<=1)
  V    [s, u] = tanh(x @ Wv)      -> matmul(lhsT=xT[f,s], rhs=Wv[f,u])
                V gets two fused ones-columns so the out-matmul also
                produces the softmax denominator (cols U:U+2).
  eST  [t, q] = exp(scale * K.T q) -> ONE fp8 DoubleRow matmul per
                (t-tile, q-block): contracts the full U=256 across the
                two uo planes of kT/qT at 2 rows/cycle.
                (tanh bounds scores to [-8, 8]; no max subtraction)
  out  [q, u] = (eST.T @ [V | 1 1]) row-normalized by column U (bf16).

Inputs are cast to bf16 on the host: halves HBM traffic (x: 8MB->4MB
per core) and the projection matmuls get Fast Weight Load.
"""

import os
import sys

import numpy as np
import ml_dtypes

for _p in ("/opt/trn_rl_repo", "/root/.axon_site/_ro/trn_rl_repo"):
    if os.path.isdir(_p) and _p not in sys.path:
        sys.path.append(_p)

import concourse.bass as bass
import concourse.mybir as mybir
import concourse.tile as tile
from concourse.bass_utils import run_bass_kernel_spmd

P = 128
B, S, F, U = 8, 2048, 1024, 256
FO, SO, UO = F // P, S // P, U // P  # 8, 16, 2
SB = 512                             # s-block width for DMA/projections
NSB = S // SB                        # 4
QB = 512                             # query-block width (free dim of eST)
NQB = S // QB                        # 4
SCALE = 1.0 / float(np.sqrt(F))      # 1/32
VW = U + 2                           # V plus fused ones columns
F32 = mybir.dt.float32
BF16 = mybir.dt.bfloat16
FP8 = mybir.dt.float8e4
DR = mybir.MatmulPerfMode.DoubleRow

NP_BF16 = ml_dtypes.bfloat16


def _split_matmul_waits(nc):
    """Walrus instruction structs have a single sem-wait slot (EventSemaphore
    has two). Peel excess waits onto NoOps (plain wait instructions on the
    same engine) inserted just before the overloaded instruction."""
    n = 0
    for bb in nc.m.functions[0].blocks:
        new_insts = []
        for inst in bb.instructions:
            cap = 2 if isinstance(inst, mybir.InstEventSemaphore) else 1
            if (
                inst.sync_info
                and inst.sync_info.on_wait
                and len(inst.sync_info.on_wait) > cap
            ):
                waits = list(inst.sync_info.on_wait)
                for w in waits[cap:]:
                    n += 1
                    nop = mybir.InstNoOp(name=f"I-xwait-{n}", ins=[], outs=[])
                    nop.engine = inst.engine
                    nop.sync_info = mybir.SyncInfo(on_wait=[w], on_update=[])
                    new_insts.append(nop)
                inst.sync_info.on_wait = waits[:cap]
            new_insts.append(inst)
        bb.instructions[:] = new_insts
    return n


def build_nc(qk_fp8=True, split_waits=True):
    qk_dt = FP8 if qk_fp8 else BF16

    nc = bass.Bass()
    # Host pre-swizzles inputs to SBUF-matching layouts so every DMA is one
    # long contiguous run per partition (8KB for x blocks, 2-4KB for weights).
    xT_d = nc.declare_dram_parameter("xT", [P, NSB, FO, SB], BF16, isOutput=False)
    w_d = {
        k: nc.declare_dram_parameter(k, [P, UO, FO, P], BF16, isOutput=False)
        for k in ("Wq", "Wk")
    }
    w_d["Wv"] = nc.declare_dram_parameter("Wv", [P, FO, U], BF16, isOutput=False)
    out_d = nc.declare_dram_parameter("out", [S, U], BF16, isOutput=True)

    TANH = mybir.ActivationFunctionType.Tanh
    EXP = mybir.ActivationFunctionType.Exp

    with tile.TileContext(nc) as tc:
        with (
            tc.tile_pool(name="wpool", bufs=1) as wpool,
            tc.tile_pool(name="qkv", bufs=1) as qkv,
            tc.tile_pool(name="smalls", bufs=1) as smalls,
            tc.tile_pool(name="recs", bufs=2) as recs,
            tc.tile_pool(name="evac", bufs=6) as evac,
            tc.tile_pool(name="exps", bufs=4) as exps,
            tc.tile_pool(name="ps_big", bufs=3, space="PSUM") as ps_big,
            tc.tile_pool(name="ps_v", bufs=2, space="PSUM") as ps_v,
            tc.tile_pool(name="ps_o", bufs=3, space="PSUM") as ps_o,
        ):
            # ---- phase 1: loads + projections. xT lives only here; its
            # SBUF space is released to the exp tiles afterwards. ----
            with tc.tile_pool(name="xpool", bufs=1) as xpool:
                # All DMAs ride the sync/SP queue: SP-issued DMAs fan out
                # over many SDMA engines, while scalar/gpsimd-issued DMAs
                # serialize on one engine (~3x slower — measured). Wq and
                # x-block 0 go first; block 0 is further split per fo chunk
                # so the first QT matmul starts as soon as possible.
                xT = xpool.tile([P, NSB, FO, SB], BF16)
                w_t = {
                    "Wq": wpool.tile([P, UO, FO, P], BF16, tag="Wq", name="w_Wq"),
                    "Wk": wpool.tile([P, UO, FO, P], BF16, tag="Wk", name="w_Wk"),
                    "Wv": wpool.tile([P, FO, U], BF16, tag="Wv", name="w_Wv"),
                }

                def dma_w(k, uo=None):
                    if uo is None:
                        nc.sync.dma_start(w_t[k][:], w_d[k][:])
                    else:
                        nc.sync.dma_start(w_t[k][:, uo], w_d[k][:, uo])

                def dma_x(sb, split=False):
                    if split:
                        for fo in range(FO):
                            nc.sync.dma_start(xT[:, sb, fo, :], xT_d[:, sb, fo, :])
                    else:
                        nc.sync.dma_start(xT[:, sb, :, :], xT_d[:, sb, :, :])

                # Byte-ordered so each consumer's data lands just in time:
                # Wq half 0 + x0-chunk 0 feed the first QT group; Wq half 1
                # right after chunk 0 (QT-uo1 re-reads resident chunks, so it
                # must not queue behind the whole x0 stream); Wk halves before
                # KT of block 0; Wv before V of block 0.
                # x0 streams in 2KB-per-partition chunks: ~1.5x the packet
                # efficiency of per-fo 1KB chunks, still fine-grained enough
                # for the projection loop to chase
                # tiny dummy DMA first: absorbs the ~0.9us queue spin-up
                # latency so the weight stream's first transfer starts
                # earlier
                dummy = smalls.tile([P, 2], BF16, tag="dummy")
                nc.sync.dma_start(dummy[:], xT_d[:, 0, 0, :2])
                dma_w("Wq", 0)
                for c in range(4):
                    nc.sync.dma_start(
                        xT[:, 0, 2 * c : 2 * c + 2, :],
                        xT_d[:, 0, 2 * c : 2 * c + 2, :],
                    )
                    if c == 0:
                        dma_w("Wq", 1)
                dma_w("Wk", 0)
                dma_w("Wk", 1)
                dma_w("Wv")
                for sb in range(1, NSB):
                    dma_x(sb)

                # PE warmup: junk matmuls on a zeroed tile keep the PE busy
                # while the x DMAs land, so HAM un-throttles and the clock
                # ramps before real work. Vector memsets the tile: it clears
                # the startup barrier ~0.4us before gpsimd does. (A memset
                # is mandatory — Tile rejects reading an unwritten tile.)
                warm = smalls.tile([P, SB], BF16, tag="warm")
                nc.vector.memset(warm[:], 0.0)
                ps_w = ps_v.tile([P, SB], F32, tag="ps_v", name="ps_w")
                for _ in range(8):
                    nc.tensor.matmul(
                        ps_w[:], warm[:, :P], warm[:], start=True, stop=True
                    )

                # ---- projections (per s-block so PE starts as DMA lands) ----
                qT = qkv.tile([P, UO, S], qk_dt, tag="qT")
                kT = qkv.tile([P, UO, S], qk_dt, tag="kT")
                vv = qkv.tile([P, SO, VW], BF16, tag="vv")
                nc.gpsimd.memset(vv[:, :, U:VW], 1.0)

                ex_tiles = [None] * NQB
                for sb in range(NSB):
                    sl = slice(sb * SB, (sb + 1) * SB)
                    for wname, dst in (("Wq", qT), ("Wk", kT)):
                        for uo in range(UO):
                            ps = ps_big.tile([P, SB], F32, tag="ps_big")
                            for fo in range(FO):
                                nc.tensor.matmul(
                                    ps[:],
                                    w_t[wname][:, uo, fo, :],
                                    xT[:, sb, fo, :],
                                    start=(fo == 0),
                                    stop=(fo == FO - 1),
                                )
                                if sb == 0 and wname == "Wq" and uo == 0 and fo:
                                    # junk filler between the chunk-paced
                                    # first group's matmuls: absorbs x0 DMA
                                    # jitter without idling the PE (an idle
                                    # PE also loses its clock ramp)
                                    nc.tensor.matmul(
                                        ps_w[:, :U],
                                        warm[:, :P],
                                        warm[:, :U],
                                        start=True,
                                        stop=True,
                                    )
                            nc.scalar.activation(dst[:, uo, sl], ps[:], TANH)
                    def emit_v(so):
                        si = (so % (SB // P)) * P
                        ps = ps_v.tile([P, U], F32, tag="ps_v")
                        for fo in range(FO):
                            nc.tensor.matmul(
                                ps[:],
                                xT[:, sb, fo, si : si + P],
                                w_t["Wv"][:, fo, :],
                                start=(fo == 0),
                                stop=(fo == FO - 1),
                            )
                        nc.scalar.activation(vv[:, so, :U], ps[:], TANH)

                    so_hi = (sb + 1) * SB // P
                    # last block: hold one V group back until after the
                    # score quad — its matmuls then hide part of the quad's
                    # serial exp-evacuation latency on the Scalar engine
                    if sb == NSB - 1:
                        so_hi -= 1
                    for so in range(sb * SB // P, so_hi):
                        emit_v(so)
                    # scores for the t-chunks this block's K just produced:
                    # fills PE gaps while the next x block's DMA lands. The
                    # needed qT q-slices come from blocks <= sb, available
                    # for qb <= sb; later qb wait for their qT (handled by
                    # Tile deps, but emitted only when ready to avoid stalls).
                    for qb in range(NQB):
                        if ex_tiles[qb] is None:
                            ex_tiles[qb] = exps.tile(
                                [P, SO, QB], BF16, tag="ex", name=f"ex{qb}"
                            )
                        if qb > sb:
                            continue
                        if sb == NSB - 1 and qb > 0:
                            # only qb0's scores stay inline: a 16-matmul DR
                            # burst here outruns the Scalar exp drain (3
                            # PSUM slots, 685ns/exp); qb1-3's move to the
                            # start of phase 2 where out-matmuls pace them
                            continue
                        qsl = slice(qb * QB, (qb + 1) * QB)
                        for to in range(sb * (SO // NSB), (sb + 1) * (SO // NSB)):
                            ps = ps_big.tile([P, QB], F32, tag="ps_big")
                            if qk_fp8:
                                nc.tensor.matmul(
                                    ps[:],
                                    kT[:, :, to * P : (to + 1) * P],
                                    qT[:, :, qsl],
                                    start=True,
                                    stop=True,
                                    perf_mode=DR,
                                )
                            else:
                                for uo in range(UO):
                                    nc.tensor.matmul(
                                        ps[:],
                                        kT[:, uo, to * P : (to + 1) * P],
                                        qT[:, uo, qsl],
                                        start=(uo == 0),
                                        stop=(uo == UO - 1),
                                    )
                            nc.scalar.activation(
                                ex_tiles[qb][:, to, :], ps[:], EXP, scale=SCALE
                            )
                    if sb == NSB - 1:
                        emit_v((sb + 1) * SB // P - 1)

            # ---- phase 2: remaining scores + output per query block.
            # Block qb's leftover scores (t-tiles from earlier s-blocks,
            # to < 4*qb) are emitted interleaved into block qb-1's output
            # groups, so their exp evacuations run on the Scalar engine
            # while the PE chews the previous block's out-matmuls. Each
            # out-group accumulates its freshest t-tiles LAST. ----
            def emit_score2(qb, to):
                ps = ps_big.tile([P, QB], F32, tag="ps_big")
                qsl = slice(qb * QB, (qb + 1) * QB)
                if qk_fp8:
                    nc.tensor.matmul(
                        ps[:],
                        kT[:, :, to * P : (to + 1) * P],
                        qT[:, :, qsl],
                        start=True,
                        stop=True,
                        perf_mode=DR,
                    )
                else:
                    for uo in range(UO):
                        nc.tensor.matmul(
                            ps[:],
                            kT[:, uo, to * P : (to + 1) * P],
                            qT[:, uo, qsl],
                            start=(uo == 0),
                            stop=(uo == UO - 1),
                        )
                nc.scalar.activation(
                    ex_tiles[qb][:, to, :], ps[:], EXP, scale=SCALE
                )

            # Deferred-score schedule, balanced so every out-group window's
            # Scalar exp load stays below its PE time:
            # - sb3-scores for qb1/2/3 lead qb0's first three out-groups
            # - pend[1] rides after groups 2-3, pend[2]+pend[3] after
            #   groups 4-11 (3,3,3,3,2,2,2,2) — each block's scores land
            #   one block ahead of its own out-groups, and the last jobs'
            #   tiles sit late enough in the next group's accumulation
            pend = {
                qb: list(range(qb * (SO // NSB))) for qb in range(NQB)
            }
            before = {g: [(g + 1, to) for to in range(SO - SO // NSB, SO)]
                      for g in range(NQB - 1)}
            after = {2: [(1, to) for to in pend[1][:2]],
                     3: [(1, to) for to in pend[1][2:]]}
            q23 = [(2, to) for to in pend[2]] + [(3, to) for to in pend[3]]
            i23 = 0
            for k, n in enumerate((3, 3, 3, 3, 2, 2, 2, 2)):
                after[4 + k] = q23[i23 : i23 + n]
                i23 += n
            g = 0
            for qb in range(NQB):
                ex = ex_tiles[qb]
                for ss in range(QB // P):
                    # the sb3 quad interleaves INSIDE this group (not read
                    # by it — it feeds a later block): one DR per four out
                    # matmuls keeps the PSUM ring ahead of the exp drain
                    il = list(before.get(g, []))
                    s0 = qb * QB + ss * P
                    ps = ps_o.tile([P, VW], F32, tag="ps_o")
                    to_order = list(range(qb * (SO // NSB), SO)) + list(
                        range(qb * (SO // NSB))
                    )
                    for n, to in enumerate(to_order):
                        nc.tensor.matmul(
                            ps[:],
                            ex[:, to, ss * P : (ss + 1) * P],
                            vv[:, to, :],
                            start=(n == 0),
                            stop=(n == SO - 1),
                        )
                        if n % 4 == 3 and il:
                            emit_score2(*il.pop(0))
                    rec = recs.tile([P, 1], F32, tag="rec")
                    nc.vector.reciprocal(rec[:], ps[:, U : U + 1])
                    ot = evac.tile([P, U], BF16, tag="ot")
                    nc.vector.tensor_scalar_mul(ot[:], ps[:, :U], rec[:])
                    nc.sync.dma_start(out_d[s0 : s0 + P, :], ot[:])
                    for job in after.get(g, []):
                        emit_score2(*job)
                    g += 1

    if split_waits:
        _split_matmul_waits(nc)
    return nc


_NC_CACHE = {}


def _get_nc(key=True):
    if key not in _NC_CACHE:
        _NC_CACHE[key] = build_nc(qk_fp8=key)
    return _NC_CACHE[key]


def _swizzle_w(w):
    # [F, U] -> [fi, fo, u]: contiguous 4KB per partition row.
    w = np.asarray(w, dtype=np.float32)
    return np.ascontiguousarray(
        w.reshape(FO, P, U).transpose(1, 0, 2).astype(NP_BF16)
    )


def _swizzle_w_halves(w):
    # [F, U] -> [fi, uo, fo, ui]: each uo half is one contiguous 2KB run
    # per partition, so it can be DMA'd independently.
    w = np.asarray(w, dtype=np.float32)
    return np.ascontiguousarray(
        w.reshape(FO, P, UO, P).transpose(1, 2, 0, 3).astype(NP_BF16)
    )


def _swizzle_x(xb):
    # [S, F] -> xT [fi, sb, fo, s]: each s-block DMA is one contiguous 8KB
    # run per partition.
    xT = np.asarray(xb, dtype=np.float32).T  # [F, S]
    return np.ascontiguousarray(
        xT.reshape(FO, P, NSB, SB).transpose(1, 2, 0, 3).astype(NP_BF16)
    )


def make_in_maps(x, Wq, Wk, Wv):
    Wq, Wk = _swizzle_w_halves(Wq), _swizzle_w_halves(Wk)
    Wv = _swizzle_w(Wv)
    return [
        {"xT": _swizzle_x(x[b]), "Wq": Wq, "Wk": Wk, "Wv": Wv}
        for b in range(B)
    ]


def kernel(x, Wq, Wk, Wv):
    nc = _get_nc()
    in_maps = make_in_maps(x, Wq, Wk, Wv)
    res = run_bass_kernel_spmd(nc, in_maps, core_ids=list(range(B)))
    return np.stack(
        [np.asarray(res.results[i]["out"], dtype=np.float32) for i in range(B)],
        axis=0,
    )


# revision 58
# speedup vs baseline: 1.0002x; 1.0002x over previous
"""Trainium2 Bass kernel for nn_AttentionTanh (B=8, S=2048, F=1024, U=256).

Data-parallel over batch: each of the 8 NeuronCores computes the full
attention for one batch example. No collectives.

Per-core dataflow (all matmuls via TensorE, out = lhsT.T @ rhs):
  xT   [F, S]  (host-swizzled bf16 input shard, F on partitions)
  QT   [u, s] = tanh(Wq.T @ x.T)  -> matmul(lhsT=Wq[f,u], rhs=xT[f,s])
  KT   [u, s] = tanh(Wk.T @ x.T)      QT/KT stored fp8e4 (scores run in
                                      fp8 DoubleRow; tanh bounds |q|<=1)
  V    [s, u] = tanh(x @ Wv)      -> matmul(lhsT=xT[f,s], rhs=Wv[f,u])
                V gets two fused ones-columns so the out-matmul also
                produces the softmax denominator (cols U:U+2).
  eST  [t, q] = exp(scale * K.T q) -> ONE fp8 DoubleRow matmul per
                (t-tile, q-block): contracts the full U=256 across the
                two uo planes of kT/qT at 2 rows/cycle.
                (tanh bounds scores to [-8, 8]; no max subtraction)
  out  [q, u] = (eST.T @ [V | 1 1]) row-normalized by column U (bf16).

Inputs are cast to bf16 on the host: halves HBM traffic (x: 8MB->4MB
per core) and the projection matmuls get Fast Weight Load.
"""

import os
import sys

import numpy as np
import ml_dtypes

for _p in ("/opt/trn_rl_repo", "/root/.axon_site/_ro/trn_rl_repo"):
    if os.path.isdir(_p) and _p not in sys.path:
        sys.path.append(_p)

import concourse.bass as bass
import concourse.mybir as mybir
import concourse.tile as tile
from concourse.bass_utils import run_bass_kernel_spmd

P = 128
B, S, F, U = 8, 2048, 1024, 256
FO, SO, UO = F // P, S // P, U // P  # 8, 16, 2
SB = 512                             # s-block width for DMA/projections
NSB = S // SB                        # 4
QB = 512                             # query-block width (free dim of eST)
NQB = S // QB                        # 4
SCALE = 1.0 / float(np.sqrt(F))      # 1/32
VW = U + 2                           # V plus fused ones columns
F32 = mybir.dt.float32
BF16 = mybir.dt.bfloat16
FP8 = mybir.dt.float8e4
DR = mybir.MatmulPerfMode.DoubleRow

NP_BF16 = ml_dtypes.bfloat16


def _split_matmul_waits(nc):
    """Walrus instruction structs have a single sem-wait slot (EventSemaphore
    has two). Peel excess waits onto NoOps (plain wait instructions on the
    same engine) inserted just before the overloaded instruction."""
    n = 0
    for bb in nc.m.functions[0].blocks:
        new_insts = []
        for inst in bb.instructions:
            cap = 2 if isinstance(inst, mybir.InstEventSemaphore) else 1
            if (
                inst.sync_info
                and inst.sync_info.on_wait
                and len(inst.sync_info.on_wait) > cap
            ):
                waits = list(inst.sync_info.on_wait)
                for w in waits[cap:]:
                    n += 1
                    nop = mybir.InstNoOp(name=f"I-xwait-{n}", ins=[], outs=[])
                    nop.engine = inst.engine
                    nop.sync_info = mybir.SyncInfo(on_wait=[w], on_update=[])
                    new_insts.append(nop)
                inst.sync_info.on_wait = waits[:cap]
            new_insts.append(inst)
        bb.instructions[:] = new_insts
    return n


def build_nc(qk_fp8=True, split_waits=True):
    qk_dt = FP8 if qk_fp8 else BF16

    nc = bass.Bass()
    # Host pre-swizzles inputs to SBUF-matching layouts so every DMA is one
    # long contiguous run per partition (8KB for x blocks, 2-4KB for weights).
    xT_d = nc.declare_dram_parameter("xT", [P, NSB, FO, SB], BF16, isOutput=False)
    w_d = {
        k: nc.declare_dram_parameter(k, [P, UO, FO, P], BF16, isOutput=False)
        for k in ("Wq", "Wk")
    }
    w_d["Wv"] = nc.declare_dram_parameter("Wv", [P, FO, U], BF16, isOutput=False)
    out_d = nc.declare_dram_parameter("out", [S, U], BF16, isOutput=True)

    TANH = mybir.ActivationFunctionType.Tanh
    EXP = mybir.ActivationFunctionType.Exp

    with tile.TileContext(nc) as tc:
        with (
            tc.tile_pool(name="wpool", bufs=1) as wpool,
            tc.tile_pool(name="qkv", bufs=1) as qkv,
            tc.tile_pool(name="smalls", bufs=1) as smalls,
            tc.tile_pool(name="recs", bufs=2) as recs,
            tc.tile_pool(name="evac", bufs=6) as evac,
            tc.tile_pool(name="exps", bufs=4) as exps,
            tc.tile_pool(name="ps_big", bufs=3, space="PSUM") as ps_big,
            tc.tile_pool(name="ps_v", bufs=2, space="PSUM") as ps_v,
            tc.tile_pool(name="ps_o", bufs=3, space="PSUM") as ps_o,
        ):
            # ---- phase 1: loads + projections. xT lives only here; its
            # SBUF space is released to the exp tiles afterwards. ----
            with tc.tile_pool(name="xpool", bufs=1) as xpool:
                # All DMAs ride the sync/SP queue: SP-issued DMAs fan out
                # over many SDMA engines, while scalar/gpsimd-issued DMAs
                # serialize on one engine (~3x slower — measured). Wq and
                # x-block 0 go first; block 0 is further split per fo chunk
                # so the first QT matmul starts as soon as possible.
                xT = xpool.tile([P, NSB, FO, SB], BF16)
                w_t = {
                    "Wq": wpool.tile([P, UO, FO, P], BF16, tag="Wq", name="w_Wq"),
                    "Wk": wpool.tile([P, UO, FO, P], BF16, tag="Wk", name="w_Wk"),
                    "Wv": wpool.tile([P, FO, U], BF16, tag="Wv", name="w_Wv"),
                }

                def dma_w(k, uo=None):
                    if uo is None:
                        nc.sync.dma_start(w_t[k][:], w_d[k][:])
                    else:
                        nc.sync.dma_start(w_t[k][:, uo], w_d[k][:, uo])

                def dma_x(sb, split=False):
                    if split:
                        for fo in range(FO):
                            nc.sync.dma_start(xT[:, sb, fo, :], xT_d[:, sb, fo, :])
                    else:
                        nc.sync.dma_start(xT[:, sb, :, :], xT_d[:, sb, :, :])

                # Byte-ordered so each consumer's data lands just in time:
                # Wq half 0 + x0-chunk 0 feed the first QT group; Wq half 1
                # right after chunk 0 (QT-uo1 re-reads resident chunks, so it
                # must not queue behind the whole x0 stream); Wk halves before
                # KT of block 0; Wv before V of block 0.
                # x0 streams in 2KB-per-partition chunks: ~1.5x the packet
                # efficiency of per-fo 1KB chunks, still fine-grained enough
                # for the projection loop to chase
                dma_w("Wq", 0)
                for c in range(4):
                    nc.sync.dma_start(
                        xT[:, 0, 2 * c : 2 * c + 2, :],
                        xT_d[:, 0, 2 * c : 2 * c + 2, :],
                    )
                    if c == 0:
                        dma_w("Wq", 1)
                dma_w("Wk", 0)
                dma_w("Wk", 1)
                dma_w("Wv")
                for sb in range(1, NSB):
                    dma_x(sb)

                # PE warmup: junk matmuls on a zeroed tile keep the PE busy
                # while the x DMAs land, so HAM un-throttles and the clock
                # ramps before real work. Vector memsets the tile: it clears
                # the startup barrier ~0.4us before gpsimd does. (A memset
                # is mandatory — Tile rejects reading an unwritten tile.)
                warm = smalls.tile([P, SB], BF16, tag="warm")
                nc.vector.memset(warm[:], 0.0)
                ps_w = ps_v.tile([P, SB], F32, tag="ps_v", name="ps_w")
                for _ in range(8):
                    nc.tensor.matmul(
                        ps_w[:], warm[:, :P], warm[:], start=True, stop=True
                    )

                # ---- projections (per s-block so PE starts as DMA lands) ----
                qT = qkv.tile([P, UO, S], qk_dt, tag="qT")
                kT = qkv.tile([P, UO, S], qk_dt, tag="kT")
                vv = qkv.tile([P, SO, VW], BF16, tag="vv")
                nc.gpsimd.memset(vv[:, :, U:VW], 1.0)

                ex_tiles = [None] * NQB
                for sb in range(NSB):
                    sl = slice(sb * SB, (sb + 1) * SB)
                    for wname, dst in (("Wq", qT), ("Wk", kT)):
                        for uo in range(UO):
                            ps = ps_big.tile([P, SB], F32, tag="ps_big")
                            for fo in range(FO):
                                nc.tensor.matmul(
                                    ps[:],
                                    w_t[wname][:, uo, fo, :],
                                    xT[:, sb, fo, :],
                                    start=(fo == 0),
                                    stop=(fo == FO - 1),
                                )
                                if sb == 0 and wname == "Wq" and uo == 0 and fo:
                                    # junk filler between the chunk-paced
                                    # first group's matmuls: absorbs x0 DMA
                                    # jitter without idling the PE (an idle
                                    # PE also loses its clock ramp)
                                    nc.tensor.matmul(
                                        ps_w[:, :U],
                                        warm[:, :P],
                                        warm[:, :U],
                                        start=True,
                                        stop=True,
                                    )
                            nc.scalar.activation(dst[:, uo, sl], ps[:], TANH)
                    def emit_v(so):
                        si = (so % (SB // P)) * P
                        ps = ps_v.tile([P, U], F32, tag="ps_v")
                        for fo in range(FO):
                            nc.tensor.matmul(
                                ps[:],
                                xT[:, sb, fo, si : si + P],
                                w_t["Wv"][:, fo, :],
                                start=(fo == 0),
                                stop=(fo == FO - 1),
                            )
                        nc.scalar.activation(vv[:, so, :U], ps[:], TANH)

                    so_hi = (sb + 1) * SB // P
                    # last block: hold one V group back until after the
                    # score quad — its matmuls then hide part of the quad's
                    # serial exp-evacuation latency on the Scalar engine
                    if sb == NSB - 1:
                        so_hi -= 1
                    for so in range(sb * SB // P, so_hi):
                        emit_v(so)
                    # scores for the t-chunks this block's K just produced:
                    # fills PE gaps while the next x block's DMA lands. The
                    # needed qT q-slices come from blocks <= sb, available
                    # for qb <= sb; later qb wait for their qT (handled by
                    # Tile deps, but emitted only when ready to avoid stalls).
                    for qb in range(NQB):
                        if ex_tiles[qb] is None:
                            ex_tiles[qb] = exps.tile(
                                [P, SO, QB], BF16, tag="ex", name=f"ex{qb}"
                            )
                        if qb > sb:
                            continue
                        if sb == NSB - 1 and qb > 0:
                            # only qb0's scores stay inline: a 16-matmul DR
                            # burst here outruns the Scalar exp drain (3
                            # PSUM slots, 685ns/exp); qb1-3's move to the
                            # start of phase 2 where out-matmuls pace them
                            continue
                        qsl = slice(qb * QB, (qb + 1) * QB)
                        for to in range(sb * (SO // NSB), (sb + 1) * (SO // NSB)):
                            ps = ps_big.tile([P, QB], F32, tag="ps_big")
                            if qk_fp8:
                                nc.tensor.matmul(
                                    ps[:],
                                    kT[:, :, to * P : (to + 1) * P],
                                    qT[:, :, qsl],
                                    start=True,
                                    stop=True,
                                    perf_mode=DR,
                                )
                            else:
                                for uo in range(UO):
                                    nc.tensor.matmul(
                                        ps[:],
                                        kT[:, uo, to * P : (to + 1) * P],
                                        qT[:, uo, qsl],
                                        start=(uo == 0),
                                        stop=(uo == UO - 1),
                                    )
                            nc.scalar.activation(
                                ex_tiles[qb][:, to, :], ps[:], EXP, scale=SCALE
                            )
                    if sb == NSB - 1:
                        emit_v((sb + 1) * SB // P - 1)

            # ---- phase 2: remaining scores + output per query block.
            # Block qb's leftover scores (t-tiles from earlier s-blocks,
            # to < 4*qb) are emitted interleaved into block qb-1's output
            # groups, so their exp evacuations run on the Scalar engine
            # while the PE chews the previous block's out-matmuls. Each
            # out-group accumulates its freshest t-tiles LAST. ----
            def emit_score2(qb, to):
                ps = ps_big.tile([P, QB], F32, tag="ps_big")
                qsl = slice(qb * QB, (qb + 1) * QB)
                if qk_fp8:
                    nc.tensor.matmul(
                        ps[:],
                        kT[:, :, to * P : (to + 1) * P],
                        qT[:, :, qsl],
                        start=True,
                        stop=True,
                        perf_mode=DR,
                    )
                else:
                    for uo in range(UO):
                        nc.tensor.matmul(
                            ps[:],
                            kT[:, uo, to * P : (to + 1) * P],
                            qT[:, uo, qsl],
                            start=(uo == 0),
                            stop=(uo == UO - 1),
                        )
                nc.scalar.activation(
                    ex_tiles[qb][:, to, :], ps[:], EXP, scale=SCALE
                )

            # Deferred-score schedule, balanced so every out-group window's
            # Scalar exp load stays below its PE time:
            # - sb3-scores for qb1/2/3 lead qb0's first three out-groups
            # - pend[1] rides after groups 2-3, pend[2]+pend[3] after
            #   groups 4-11 (3,3,3,3,2,2,2,2) — each block's scores land
            #   one block ahead of its own out-groups, and the last jobs'
            #   tiles sit late enough in the next group's accumulation
            pend = {
                qb: list(range(qb * (SO // NSB))) for qb in range(NQB)
            }
            before = {g: [(g + 1, to) for to in range(SO - SO // NSB, SO)]
                      for g in range(NQB - 1)}
            after = {2: [(1, to) for to in pend[1][:2]],
                     3: [(1, to) for to in pend[1][2:]]}
            q23 = [(2, to) for to in pend[2]] + [(3, to) for to in pend[3]]
            i23 = 0
            for k, n in enumerate((3, 3, 3, 3, 2, 2, 2, 2)):
                after[4 + k] = q23[i23 : i23 + n]
                i23 += n
            g = 0
            for qb in range(NQB):
                ex = ex_tiles[qb]
                for ss in range(QB // P):
                    # the sb3 quad interleaves INSIDE this group (not read
                    # by it — it feeds a later block): one DR per four out
                    # matmuls keeps the PSUM ring ahead of the exp drain
                    il = list(before.get(g, []))
                    s0 = qb * QB + ss * P
                    ps = ps_o.tile([P, VW], F32, tag="ps_o")
                    to_order = list(range(qb * (SO // NSB), SO)) + list(
                        range(qb * (SO // NSB))
                    )
                    for n, to in enumerate(to_order):
                        nc.tensor.matmul(
                            ps[:],
                            ex[:, to, ss * P : (ss + 1) * P],
                            vv[:, to, :],
                            start=(n == 0),
                            stop=(n == SO - 1),
                        )
                        if n % 4 == 3 and il:
                            emit_score2(*il.pop(0))
                    rec = recs.tile([P, 1], F32, tag="rec")
                    nc.vector.reciprocal(rec[:], ps[:, U : U + 1])
                    ot = evac.tile([P, U], BF16, tag="ot")
                    nc.vector.tensor_scalar_mul(ot[:], ps[:, :U], rec[:])
                    nc.sync.dma_start(out_d[s0 : s0 + P, :], ot[:])
                    for job in after.get(g, []):
                        emit_score2(*job)
                    g += 1

    if split_waits:
        _split_matmul_waits(nc)
    return nc


_NC_CACHE = {}


def _get_nc(key=True):
    if key not in _NC_CACHE:
        _NC_CACHE[key] = build_nc(qk_fp8=key)
    return _NC_CACHE[key]


def _swizzle_w(w):
    # [F, U] -> [fi, fo, u]: contiguous 4KB per partition row.
    w = np.asarray(w, dtype=np.float32)
    return np.ascontiguousarray(
        w.reshape(FO, P, U).transpose(1, 0, 2).astype(NP_BF16)
    )


def _swizzle_w_halves(w):
    # [F, U] -> [fi, uo, fo, ui]: each uo half is one contiguous 2KB run
    # per partition, so it can be DMA'd independently.
    w = np.asarray(w, dtype=np.float32)
    return np.ascontiguousarray(
        w.reshape(FO, P, UO, P).transpose(1, 2, 0, 3).astype(NP_BF16)
    )


def _swizzle_x(xb):
    # [S, F] -> xT [fi, sb, fo, s]: each s-block DMA is one contiguous 8KB
    # run per partition.
    xT = np.asarray(xb, dtype=np.float32).T  # [F, S]
    return np.ascontiguousarray(
        xT.reshape(FO, P, NSB, SB).transpose(1, 2, 0, 3).astype(NP_BF16)
    )


def make_in_maps(x, Wq, Wk, Wv):
    Wq, Wk = _swizzle_w_halves(Wq), _swizzle_w_halves(Wk)
    Wv = _swizzle_w(Wv)
    return [
        {"xT": _swizzle_x(x[b]), "Wq": Wq, "Wk": Wk, "Wv": Wv}
        for b in range(B)
    ]


def kernel(x, Wq, Wk, Wv):
    nc = _get_nc()
    in_maps = make_in_maps(x, Wq, Wk, Wv)
    res = run_bass_kernel_spmd(nc, in_maps, core_ids=list(range(B)))
    return np.stack(
        [np.asarray(res.results[i]["out"], dtype=np.float32) for i in range(B)],
        axis=0,
    )


# revision 60
# speedup vs baseline: 1.1847x; 1.1845x over previous
"""Trainium2 Bass kernel for nn_AttentionTanh (B=8, S=2048, F=1024, U=256).

Data-parallel over batch: each of the 8 NeuronCores computes the full
attention for one batch example. No collectives.

Per-core dataflow (all matmuls via TensorE, out = lhsT.T @ rhs):
  xT   [F, S]  (host-swizzled bf16 input shard, F on partitions)
  QT   [u, s] = tanh(Wq.T @ x.T)  -> matmul(lhsT=Wq[f,u], rhs=xT[f,s])
  KT   [u, s] = tanh(Wk.T @ x.T)      QT/KT stored fp8e4 (scores run in
                                      fp8 DoubleRow; tanh bounds |q|<=1)
  V    [s, u] = tanh(x @ Wv)      -> matmul(lhsT=xT[f,s], rhs=Wv[f,u])
                V gets two fused ones-columns so the out-matmul also
                produces the softmax denominator (cols U:U+2).
  eST  [t, q] = exp(scale * K.T q) -> ONE fp8 DoubleRow matmul per
                (t-tile, q-block): contracts the full U=256 across the
                two uo planes of kT/qT at 2 rows/cycle.
                (tanh bounds scores to [-8, 8]; no max subtraction)
  out  [q, u] = (eST.T @ [V | 1 1]) row-normalized by column U (bf16).

Inputs are cast to bf16 on the host: halves HBM traffic (x: 8MB->4MB
per core) and the projection matmuls get Fast Weight Load.
"""

import os
import sys

import numpy as np
import ml_dtypes

for _p in ("/opt/trn_rl_repo", "/root/.axon_site/_ro/trn_rl_repo"):
    if os.path.isdir(_p) and _p not in sys.path:
        sys.path.append(_p)

import concourse.bass as bass
import concourse.mybir as mybir
import concourse.tile as tile
from concourse.bass_utils import run_bass_kernel_spmd

P = 128
B, S, F, U = 8, 2048, 1024, 256
FO, SO, UO = F // P, S // P, U // P  # 8, 16, 2
SB = 512                             # s-block width for DMA/projections
NSB = S // SB                        # 4
QB = 512                             # query-block width (free dim of eST)
NQB = S // QB                        # 4
SCALE = 1.0 / float(np.sqrt(F))      # 1/32
VW = U + 2                           # V plus fused ones columns
F32 = mybir.dt.float32
BF16 = mybir.dt.bfloat16
FP8 = mybir.dt.float8e4
DR = mybir.MatmulPerfMode.DoubleRow

NP_BF16 = ml_dtypes.bfloat16


def _split_matmul_waits(nc):
    """Walrus instruction structs have a single sem-wait slot (EventSemaphore
    has two). Peel excess waits onto NoOps (plain wait instructions on the
    same engine) inserted just before the overloaded instruction."""
    n = 0
    for bb in nc.m.functions[0].blocks:
        new_insts = []
        for inst in bb.instructions:
            cap = 2 if isinstance(inst, mybir.InstEventSemaphore) else 1
            if (
                inst.sync_info
                and inst.sync_info.on_wait
                and len(inst.sync_info.on_wait) > cap
            ):
                waits = list(inst.sync_info.on_wait)
                for w in waits[cap:]:
                    n += 1
                    nop = mybir.InstNoOp(name=f"I-xwait-{n}", ins=[], outs=[])
                    nop.engine = inst.engine
                    nop.sync_info = mybir.SyncInfo(on_wait=[w], on_update=[])
                    new_insts.append(nop)
                inst.sync_info.on_wait = waits[:cap]
            new_insts.append(inst)
        bb.instructions[:] = new_insts
    return n


def build_nc(qk_fp8=True, split_waits=True):
    qk_dt = FP8 if qk_fp8 else BF16

    nc = bass.Bass()
    # Host pre-swizzles inputs to SBUF-matching layouts so every DMA is one
    # long contiguous run per partition (8KB for x blocks, 2-4KB for weights).
    xT_d = nc.declare_dram_parameter("xT", [P, NSB, FO, SB], BF16, isOutput=False)
    w_d = {
        k: nc.declare_dram_parameter(k, [P, UO, FO, P], BF16, isOutput=False)
        for k in ("Wq", "Wk")
    }
    w_d["Wv"] = nc.declare_dram_parameter("Wv", [P, FO, U], BF16, isOutput=False)
    out_d = nc.declare_dram_parameter("out", [S, U], BF16, isOutput=True)

    TANH = mybir.ActivationFunctionType.Tanh
    EXP = mybir.ActivationFunctionType.Exp

    with tile.TileContext(nc) as tc:
        with (
            tc.tile_pool(name="wpool", bufs=1) as wpool,
            tc.tile_pool(name="qkv", bufs=1) as qkv,
            tc.tile_pool(name="smalls", bufs=1) as smalls,
            tc.tile_pool(name="recs", bufs=2) as recs,
            tc.tile_pool(name="evac", bufs=6) as evac,
            tc.tile_pool(name="exps", bufs=4) as exps,
            tc.tile_pool(name="ps_big", bufs=3, space="PSUM") as ps_big,
            tc.tile_pool(name="ps_v", bufs=2, space="PSUM") as ps_v,
            tc.tile_pool(name="ps_o", bufs=3, space="PSUM") as ps_o,
        ):
            # ---- phase 1: loads + projections. xT lives only here; its
            # SBUF space is released to the exp tiles afterwards. ----
            with tc.tile_pool(name="xpool", bufs=1) as xpool:
                # All DMAs ride the sync/SP queue: SP-issued DMAs fan out
                # over many SDMA engines, while scalar/gpsimd-issued DMAs
                # serialize on one engine (~3x slower — measured). Wq and
                # x-block 0 go first; block 0 is further split per fo chunk
                # so the first QT matmul starts as soon as possible.
                xT = xpool.tile([P, NSB, FO, SB], BF16)
                w_t = {
                    "Wq": wpool.tile([P, UO, FO, P], BF16, tag="Wq", name="w_Wq"),
                    "Wk": wpool.tile([P, UO, FO, P], BF16, tag="Wk", name="w_Wk"),
                    "Wv": wpool.tile([P, FO, U], BF16, tag="Wv", name="w_Wv"),
                }

                def dma_w(k, uo=None):
                    if uo is None:
                        nc.sync.dma_start(w_t[k][:], w_d[k][:])
                    else:
                        nc.sync.dma_start(w_t[k][:, uo], w_d[k][:, uo])

                def dma_x(sb, split=False):
                    if split:
                        for fo in range(FO):
                            nc.sync.dma_start(xT[:, sb, fo, :], xT_d[:, sb, fo, :])
                    else:
                        nc.sync.dma_start(xT[:, sb, :, :], xT_d[:, sb, :, :])

                # Byte-ordered so each consumer's data lands just in time:
                # Wq half 0 + x0-chunk 0 feed the first QT group; Wq half 1
                # right after chunk 0 (QT-uo1 re-reads resident chunks, so it
                # must not queue behind the whole x0 stream); Wk halves before
                # KT of block 0; Wv before V of block 0.
                # x0 streams in 2KB-per-partition chunks: ~1.5x the packet
                # efficiency of per-fo 1KB chunks, still fine-grained enough
                # for the projection loop to chase
                dma_w("Wq", 0)
                for c in range(4):
                    nc.sync.dma_start(
                        xT[:, 0, 2 * c : 2 * c + 2, :],
                        xT_d[:, 0, 2 * c : 2 * c + 2, :],
                    )
                    if c == 0:
                        dma_w("Wq", 1)
                dma_w("Wk", 0)
                dma_w("Wk", 1)
                dma_w("Wv")
                for sb in range(1, NSB):
                    dma_x(sb)

                # PE warmup: junk matmuls on a zeroed tile keep the PE busy
                # while the x DMAs land, so HAM un-throttles and the clock
                # ramps before real work. Vector memsets the tile: it clears
                # the startup barrier ~0.4us before gpsimd does. (A memset
                # is mandatory — Tile rejects reading an unwritten tile.)
                warm = smalls.tile([P, SB], BF16, tag="warm")
                nc.vector.memset(warm[:], 0.0)
                ps_w = ps_v.tile([P, SB], F32, tag="ps_v", name="ps_w")
                for _ in range(8):
                    nc.tensor.matmul(
                        ps_w[:], warm[:, :P], warm[:], start=True, stop=True
                    )

                # ---- projections (per s-block so PE starts as DMA lands) ----
                qT = qkv.tile([P, UO, S], qk_dt, tag="qT")
                kT = qkv.tile([P, UO, S], qk_dt, tag="kT")
                vv = qkv.tile([P, SO, VW], BF16, tag="vv")
                nc.gpsimd.memset(vv[:, :, U:VW], 1.0)

                ex_tiles = [None] * NQB
                for sb in range(NSB):
                    sl = slice(sb * SB, (sb + 1) * SB)
                    for wname, dst in (("Wq", qT), ("Wk", kT)):
                        for uo in range(UO):
                            ps = ps_big.tile([P, SB], F32, tag="ps_big")
                            for fo in range(FO):
                                nc.tensor.matmul(
                                    ps[:],
                                    w_t[wname][:, uo, fo, :],
                                    xT[:, sb, fo, :],
                                    start=(fo == 0),
                                    stop=(fo == FO - 1),
                                )
                                if sb == 0 and wname == "Wq" and uo == 0 and fo:
                                    # junk filler between the chunk-paced
                                    # first group's matmuls: absorbs x0 DMA
                                    # jitter without idling the PE (an idle
                                    # PE also loses its clock ramp)
                                    nc.tensor.matmul(
                                        ps_w[:, :U],
                                        warm[:, :P],
                                        warm[:, :U],
                                        start=True,
                                        stop=True,
                                    )
                            nc.scalar.activation(dst[:, uo, sl], ps[:], TANH)
                    def emit_v(so):
                        si = (so % (SB // P)) * P
                        ps = ps_v.tile([P, U], F32, tag="ps_v")
                        for fo in range(FO):
                            nc.tensor.matmul(
                                ps[:],
                                xT[:, sb, fo, si : si + P],
                                w_t["Wv"][:, fo, :],
                                start=(fo == 0),
                                stop=(fo == FO - 1),
                            )
                        nc.scalar.activation(vv[:, so, :U], ps[:], TANH)

                    so_hi = (sb + 1) * SB // P
                    # last block: hold one V group back until after the
                    # score quad — its matmuls then hide part of the quad's
                    # serial exp-evacuation latency on the Scalar engine
                    if sb == NSB - 1:
                        so_hi -= 1
                    for so in range(sb * SB // P, so_hi):
                        emit_v(so)
                    # scores for the t-chunks this block's K just produced:
                    # fills PE gaps while the next x block's DMA lands. The
                    # needed qT q-slices come from blocks <= sb, available
                    # for qb <= sb; later qb wait for their qT (handled by
                    # Tile deps, but emitted only when ready to avoid stalls).
                    for qb in range(NQB):
                        if ex_tiles[qb] is None:
                            ex_tiles[qb] = exps.tile(
                                [P, SO, QB], BF16, tag="ex", name=f"ex{qb}"
                            )
                        if qb > sb:
                            continue
                        if sb == NSB - 1 and qb > 0:
                            # only qb0's scores stay inline: a 16-matmul DR
                            # burst here outruns the Scalar exp drain (3
                            # PSUM slots, 685ns/exp); qb1-3's move to the
                            # start of phase 2 where out-matmuls pace them
                            continue
                        qsl = slice(qb * QB, (qb + 1) * QB)
                        for to in range(sb * (SO // NSB), (sb + 1) * (SO // NSB)):
                            ps = ps_big.tile([P, QB], F32, tag="ps_big")
                            if qk_fp8:
                                nc.tensor.matmul(
                                    ps[:],
                                    kT[:, :, to * P : (to + 1) * P],
                                    qT[:, :, qsl],
                                    start=True,
                                    stop=True,
                                    perf_mode=DR,
                                )
                            else:
                                for uo in range(UO):
                                    nc.tensor.matmul(
                                        ps[:],
                                        kT[:, uo, to * P : (to + 1) * P],
                                        qT[:, uo, qsl],
                                        start=(uo == 0),
                                        stop=(uo == UO - 1),
                                    )
                            nc.scalar.activation(
                                ex_tiles[qb][:, to, :], ps[:], EXP, scale=SCALE
                            )
                    if sb == NSB - 1:
                        emit_v((sb + 1) * SB // P - 1)

            # ---- phase 2: remaining scores + output per query block.
            # Block qb's leftover scores (t-tiles from earlier s-blocks,
            # to < 4*qb) are emitted interleaved into block qb-1's output
            # groups, so their exp evacuations run on the Scalar engine
            # while the PE chews the previous block's out-matmuls. Each
            # out-group accumulates its freshest t-tiles LAST. ----
            def emit_score2(qb, to):
                ps = ps_big.tile([P, QB], F32, tag="ps_big")
                qsl = slice(qb * QB, (qb + 1) * QB)
                if qk_fp8:
                    nc.tensor.matmul(
                        ps[:],
                        kT[:, :, to * P : (to + 1) * P],
                        qT[:, :, qsl],
                        start=True,
                        stop=True,
                        perf_mode=DR,
                    )
                else:
                    for uo in range(UO):
                        nc.tensor.matmul(
                            ps[:],
                            kT[:, uo, to * P : (to + 1) * P],
                            qT[:, uo, qsl],
                            start=(uo == 0),
                            stop=(uo == UO - 1),
                        )
                nc.scalar.activation(
                    ex_tiles[qb][:, to, :], ps[:], EXP, scale=SCALE
                )

            # Deferred-score schedule, balanced so every out-group window's
            # Scalar exp load stays below its PE time:
            # - sb3-scores for qb1/2/3 lead qb0's first three out-groups
            # - pend[1] rides after groups 2-3, pend[2]+pend[3] after
            #   groups 4-11 (3,3,3,3,2,2,2,2) — each block's scores land
            #   one block ahead of its own out-groups, and the last jobs'
            #   tiles sit late enough in the next group's accumulation
            pend = {
                qb: list(range(qb * (SO // NSB))) for qb in range(NQB)
            }
            before = {g: [(g + 1, to) for to in range(SO - SO // NSB, SO)]
                      for g in range(NQB - 1)}
            after = {2: [(1, to) for to in pend[1][:2]],
                     3: [(1, to) for to in pend[1][2:]]}
            q23 = [(2, to) for to in pend[2]] + [(3, to) for to in pend[3]]
            i23 = 0
            for k, n in enumerate((3, 3, 3, 3, 2, 2, 2, 2)):
                after[4 + k] = q23[i23 : i23 + n]
                i23 += n
            g = 0
            for qb in range(NQB):
                ex = ex_tiles[qb]
                for ss in range(QB // P):
                    # the sb3 quad interleaves INSIDE this group (not read
                    # by it — it feeds a later block): one DR per four out
                    # matmuls keeps the PSUM ring ahead of the exp drain
                    il = list(before.get(g, []))
                    s0 = qb * QB + ss * P
                    ps = ps_o.tile([P, VW], F32, tag="ps_o")
                    to_order = list(range(qb * (SO // NSB), SO)) + list(
                        range(qb * (SO // NSB))
                    )
                    for n, to in enumerate(to_order):
                        nc.tensor.matmul(
                            ps[:],
                            ex[:, to, ss * P : (ss + 1) * P],
                            vv[:, to, :],
                            start=(n == 0),
                            stop=(n == SO - 1),
                        )
                        if n % 4 == 3 and il:
                            emit_score2(*il.pop(0))
                    rec = recs.tile([P, 1], F32, tag="rec")
                    nc.vector.reciprocal(rec[:], ps[:, U : U + 1])
                    ot = evac.tile([P, U], BF16, tag="ot")
                    nc.vector.tensor_scalar_mul(ot[:], ps[:, :U], rec[:])
                    nc.sync.dma_start(out_d[s0 : s0 + P, :], ot[:])
                    for job in after.get(g, []):
                        emit_score2(*job)
                    g += 1

    if split_waits:
        _split_matmul_waits(nc)
    _strip_unreferenced_mm_updates(nc)
    return nc


_NC_CACHE = {}


def _get_nc(key=True):
    if key not in _NC_CACHE:
        _NC_CACHE[key] = build_nc(qk_fp8=key)
    return _NC_CACHE[key]


def _swizzle_w(w):
    # [F, U] -> [fi, fo, u]: contiguous 4KB per partition row.
    w = np.asarray(w, dtype=np.float32)
    return np.ascontiguousarray(
        w.reshape(FO, P, U).transpose(1, 0, 2).astype(NP_BF16)
    )


def _swizzle_w_halves(w):
    # [F, U] -> [fi, uo, fo, ui]: each uo half is one contiguous 2KB run
    # per partition, so it can be DMA'd independently.
    w = np.asarray(w, dtype=np.float32)
    return np.ascontiguousarray(
        w.reshape(FO, P, UO, P).transpose(1, 2, 0, 3).astype(NP_BF16)
    )


def _swizzle_x(xb):
    # [S, F] -> xT [fi, sb, fo, s]: each s-block DMA is one contiguous 8KB
    # run per partition.
    xT = np.asarray(xb, dtype=np.float32).T  # [F, S]
    return np.ascontiguousarray(
        xT.reshape(FO, P, NSB, SB).transpose(1, 2, 0, 3).astype(NP_BF16)
    )


def make_in_maps(x, Wq, Wk, Wv):
    Wq, Wk = _swizzle_w_halves(Wq), _swizzle_w_halves(Wk)
    Wv = _swizzle_w(Wv)
    return [
        {"xT": _swizzle_x(x[b]), "Wq": Wq, "Wk": Wk, "Wv": Wv}
        for b in range(B)
    ]


def kernel(x, Wq, Wk, Wv):
    nc = _get_nc()
    in_maps = make_in_maps(x, Wq, Wk, Wv)
    res = run_bass_kernel_spmd(nc, in_maps, core_ids=list(range(B)))
    return np.stack(
        [np.asarray(res.results[i]["out"], dtype=np.float32) for i in range(B)],
        axis=0,
    )


def _strip_unreferenced_mm_updates(nc):
    """Matmuls all increment one completion counter; waits reference absolute
    counts. On the in-order PE, an increment is only needed where some wait
    targets that exact position. Strip the rest and renumber every wait by
    the surviving prefix count (saves per-instruction sem-update time)."""
    from collections import Counter
    insts = [i for bb in nc.m.functions[0].blocks for i in bb.instructions]
    mms = [i for i in insts if isinstance(i, mybir.InstMatmult)
           and i.sync_info and i.sync_info.on_update]
    if not mms:
        return 0
    c = Counter(u.id for i in mms for u in i.sync_info.on_update)
    sem = c.most_common(1)[0][0]
    mm_pos = [i for i in mms
              if any(u.id == sem for u in i.sync_info.on_update)]
    vals = set()
    for i in insts:
        if i.sync_info and i.sync_info.on_wait:
            for w in i.sync_info.on_wait:
                if w.id == sem:
                    if not w.uses_immediate:
                        return 0  # register-based wait: bail out entirely
                    vals.add(w.wait_value)
    keep = [(p + 1) in vals for p in range(len(mm_pos))]
    prefix = [0] * (len(mm_pos) + 1)
    for p in range(len(mm_pos)):
        prefix[p + 1] = prefix[p] + (1 if keep[p] else 0)
    n = 0
    for p, i in enumerate(mm_pos):
        if not keep[p]:
            i.sync_info.on_update = [
                u for u in i.sync_info.on_update if u.id != sem
            ]
            n += 1
    for i in insts:
        if i.sync_info and i.sync_info.on_wait:
            for w in i.sync_info.on_wait:
                if w.id == sem:
                    w.wait_value = prefix[min(w.wait_value, len(mm_pos))]
    return n
